# revision 32
# baseline (speedup 1.0000x reference)
"""Multi-head causal attention (B=4, T=2048, E=1024, H=16, D=64) on 8 trn2
NeuronCores via Bass/Tile.

Sharding: core c handles batch b = c//2 and heads [half*8, half*8+8), half =
c%2. Each core computes its 8 heads' attention and a partial output
projection; the host sums the two half partials per batch, transposes, and
adds the bias.

On-device layout is "transposed": activations are [feature, token] so every
matmul contracts over the partition dim. Softmax denominators come from a
ones-column appended to the stationary V operand (M=65 matmuls); masking is
applied block-wise (128x128) with patterns derived from the actual mask input
at build time. No max-subtraction is needed: scores are ~N(0, 0.083^2).

This version software-pipelines the whole kernel: the attention i-loop is
ACT(exp)-paced, so projection matmuls for the next t-tile, output-projection
matmuls for the previous t-tile, and softmax-tail work are injected as
"filler" closures between attention steps to keep the PE busy. Inputs are
pre-tiled host-side so every DMA moves one contiguous 128KB block.
"""
import numpy as np
import ml_dtypes
from collections import deque
from contextlib import ExitStack

import concourse.bass as bass
import concourse.mybir as mybir
import concourse.tile as tile
from concourse.bass_utils import run_bass_kernel_spmd
from concourse.vector_clock import ScopedClock

BF16 = mybir.dt.bfloat16
F32 = mybir.dt.float32
NPBF16 = ml_dtypes.bfloat16
NPFP8 = ml_dtypes.float8_e4m3fn

B, T, E, H, D = 4, 2048, 1024, 16, 64
HPC = 8            # heads per core
DC = HPC * D       # 512: stacked head dim per core
TJ = 512           # t tile (matmul free dim)
NJ = T // TJ       # 4
SI = 128           # s tile (psum partition dim)
NSI = T // SI      # 16
EC = E // 128      # 8 e-chunks
NP = HPC // 2      # 4 head pairs

# ---------------------------------------------------------------------------
# Workarounds for this walrus build: at most ONE sync wait per instruction.
# ---------------------------------------------------------------------------
_PATCHED = False


def _patched_drain_and_barrier(self, tick_clock, wait_clock):
    drain_inst = self.nc.sync.drain(fusable=False)
    wait_clock.add_sem_waits(
        drain_inst.ins, ScopedClock({None: tick_clock.global_clock})
    )
    si = drain_inst.ins.sync_info
    if si is not None and len(si.on_wait) > 1:
        waits = list(si.on_wait)
        drain_inst.ins.sync_info = mybir.SyncInfo(
            on_wait=waits[:1], on_update=list(si.on_update)
        )
        for ofs in range(1, len(waits)):
            extra = self.nc.sync.drain(fusable=False)
            extra.ins.sync_info = mybir.SyncInfo(
                on_wait=waits[ofs : ofs + 1], on_update=[]
            )
    self.nc.all_engine_barrier()
    assert self.sems is not None
    popped = self.nc._tile_sem_poison_stack.pop()
    assert popped is self._sem_poison
    self.nc.clear_and_free_semaphores(list(self.sems.allocated().values()))
    self.nc.all_engine_barrier()


def _install_patches():
    global _PATCHED
    if _PATCHED:
        return
    tile.TileContext._drain_and_barrier = _patched_drain_and_barrier
    _PATCHED = True


def _make_carrier(nc, engine, wait):
    """Wait-only EventSemaphore on `engine` (cheap: ~70ns, no pipe flush)."""
    ev = mybir.InstEventSemaphore(name=f"W-{nc.next_id()}", ins=[], outs=[])
    ev.engine = engine
    ev.sync_info = mybir.SyncInfo(on_wait=[wait], on_update=[])
    return ev


_ENGINE_SEM = {
    "EngineType.PE": "PE",
    "EngineType.DVE": "DVE",
    "EngineType.Activation": "Activation",
    "EngineType.SP": "SP",
    "EngineType.Pool": "Pool",
}
# engines with in-order issue AND in-order completion for these inst types:
# a wait on the engine's own completion sem is redundant. Ldweights excluded
# (the PE reorder window pulls it ahead of in-flight matmuls).
_DROPPABLE = (
    "InstMatmult", "InstActivation", "InstTensorTensor", "InstTensorCopy",
    "InstTensorReduce", "InstMemset", "InstReciprocal", "InstDMACopy",
    "InstCopyPredicated", "InstTensorScalarPtr", "InstTensorScalar",
    "InstCast", "InstDveOp", "InstCustomDve",
)


def _split_multi_waits(nc):
    for bbw in list(nc.bb_map.values()):
        bb = bbw.bb
        insts = bb.instructions
        if not any(
            getattr(i, "sync_info", None) is not None and len(i.sync_info.on_wait) > 1
            for i in insts
        ):
            continue
        out = []
        for inst in insts:
            si = getattr(inst, "sync_info", None)
            waits = list(si.on_wait) if si is not None else []
            if len(waits) > 1:
                own = _ENGINE_SEM.get(str(inst.engine))
                tn = type(inst).__name__
                if own is not None and tn.startswith(_DROPPABLE):
                    waits = [
                        w for w in waits
                        if w.ant_name.rsplit("_", 1)[0] != own
                    ] or waits[-1:]
            if len(waits) > 1:
                for w in waits[:-1]:
                    out.append(_make_carrier(nc, inst.engine, w))
                waits = waits[-1:]
            if si is not None and list(si.on_wait) != waits:
                inst.sync_info = mybir.SyncInfo(
                    on_wait=waits, on_update=list(si.on_update)
                )
            out.append(inst)
        insts[:] = out


# ---------------------------------------------------------------------------
# Mask analysis (host side, 128x128 blocks).
# ---------------------------------------------------------------------------
def _classify_mask(mask):
    """mask: [T, T] bool, mask[t, s]=True means masked (score -> -inf).

    Returns (btab, patterns): btab[i][jj] in {'skip', 'dense', int u};
    patterns[u] is a [128,128] bf16 multiplier in [s, t] orientation."""
    nb = T // 128
    m = np.asarray(mask, dtype=bool)
    patterns = []
    index = {}
    btab = [[None] * nb for _ in range(nb)]
    for i in range(nb):          # s block
        for jj in range(nb):     # t block
            sub = m[jj * 128 : (jj + 1) * 128, i * 128 : (i + 1) * 128]  # [t, s]
            if sub.all():
                btab[i][jj] = "skip"
            elif not sub.any():
                btab[i][jj] = "dense"
            else:
                pat = (~sub).T.astype(NPBF16)  # [s, t] multiplier
                key = pat.tobytes()
                if key not in index:
                    index[key] = len(patterns)
                    patterns.append(pat)
                btab[i][jj] = index[key]
    if not patterns:
        patterns.append(np.ones((128, 128), NPBF16))
    return btab, np.stack(patterns)


# ---------------------------------------------------------------------------
# Kernel builder (SPMD program, identical on all 8 cores).
# ---------------------------------------------------------------------------
def _build(btab, n_pat):
    nc = bass.Bass()
    FP8 = mybir.dt.float8e4
    # pre-tiled streams: tile (j, e) is rows (j*EC+e)*128 .. +128, contiguous
    # q/k are fp8 (DoubleRow matmuls; errors attenuate through softmax)
    qTt = nc.declare_dram_parameter("qTt", [128, NJ * EC * TJ], FP8, isOutput=False)
    kTt = nc.declare_dram_parameter("kTt", [128, NJ * EC * TJ], FP8, isOutput=False)
    vTt = nc.declare_dram_parameter("vTt", [128, NJ * EC * TJ], BF16, isOutput=False)
    wq = nc.declare_dram_parameter("wq", [128, EC * DC], FP8, isOutput=False)
    wk = nc.declare_dram_parameter("wk", [128, EC * DC], FP8, isOutput=False)
    wv = nc.declare_dram_parameter("wv", [128, EC * DC], BF16, isOutput=False)
    wpT = nc.declare_dram_parameter("wpT", [DC, E], BF16, isOutput=False)
    pat = nc.declare_dram_parameter("pat", [n_pat * 128, 128], BF16, isOutput=False)
    selexp = nc.declare_dram_parameter("selexp", [128, 256], BF16, isOutput=False)
    # pre-tiled output: tile (j, m) is rows (j*EC+m)*128 .. +128
    yTt = nc.declare_dram_parameter("yTt", [NJ * EC * 128, TJ], F32, isOutput=True)

    EXP = mybir.ActivationFunctionType.Exp
    LN = mybir.ActivationFunctionType.Ln

    with ExitStack() as ctx:
        tc = ctx.enter_context(tile.TileContext(nc))
        # SBUF pools
        consts = ctx.enter_context(tc.tile_pool(name="consts", bufs=1))
        streams = ctx.enter_context(tc.tile_pool(name="streams", bufs=1))
        acts = ctx.enter_context(tc.tile_pool(name="acts", bufs=1))
        work = ctx.enter_context(tc.tile_pool(name="work", bufs=1))
        # PSUM pools
        psA = ctx.enter_context(tc.tile_pool(name="psA", bufs=1, space="PSUM"))
        psB = ctx.enter_context(tc.tile_pool(name="psB", bufs=1, space="PSUM"))

        # ---- constants ----
        # weights as single partition-major tiles; e-chunk c lives at
        # cols [c*DC, (c+1)*DC)
        wq_sb = consts.tile([128, EC * DC], FP8, tag="wq", name="wq", bufs=1)
        wk_sb = consts.tile([128, EC * DC], FP8, tag="wk", name="wk", bufs=1)
        wv_sb = consts.tile([128, EC * DC], BF16, tag="wv", name="wv", bufs=1)
        wp_sb = [consts.tile([128, E], BF16, tag=f"wp{p}", name=f"wp{p}", bufs=1) for p in range(NP)]
        pat_sb = [consts.tile([128, 128], BF16, tag=f"pat{u}", name=f"pat{u}", bufs=1) for u in range(n_pat)]
        selexp_sb = consts.tile([128, 256], BF16, tag="selexp", name="selexp", bufs=1)
        dummy_sb = consts.tile([1, 2], BF16, tag="dummy", name="dummy", bufs=1)

        # ---- persistent activations (per (p, j) tiles so pipelined writes
        # to tile j+1 never alias reads of tile j) ----
        xq_t = [[acts.tile([128, TJ], BF16, tag=f"xq{p}_{j}", name=f"xq{p}_{j}", bufs=1)
                 for j in range(NJ)] for p in range(NP)]
        xk_t = [[acts.tile([128, TJ], BF16, tag=f"xk{p}_{j}", name=f"xk{p}_{j}", bufs=1)
                 for j in range(NJ)] for p in range(NP)]
        # xv tiles: per s-tile, heads laid out as 8 x (64 cols xv | 1 col ones)
        xv_sb = [acts.tile([128, HPC * 65], BF16, tag=f"xv{i}", name=f"xv{i}", bufs=1) for i in range(NSI)]
        osc_sb_all = [
            [acts.tile([128, TJ], BF16, tag=f"osc{p}_{jj}", name=f"osc{p}_{jj}", bufs=1)
             for p in range(NP)]
            for jj in range(2)
        ]

        # ---------------------------------------------------------------
        # Filler queue: closures emitting ~1-2 PE matmuls (+ their DVE/DMA
        # tails). `require(key)` force-drains through a named closure.
        # ---------------------------------------------------------------
        fillers = deque()        # entries: (cost, fn)
        ready_idx = {}           # key -> push counter of last closure for key
        drained = [0]            # count of executed closures
        pushed = [0]
        debt = [0.0]

        def push(cost, fn, key=None):
            fillers.append((cost, fn))
            pushed[0] += 1
            if key is not None:
                ready_idx[key] = pushed[0]

        def _run_one():
            cost, fn = fillers.popleft()
            fn()
            drained[0] += 1
            return cost

        def pump(units):
            debt[0] += units
            while fillers and debt[0] >= fillers[0][0]:
                debt[0] -= _run_one()

        def require(key):
            idx = ready_idx.get(key, 0)
            while drained[0] < idx:
                _run_one()

        # ---------------------------------------------------------------
        # DMA emission
        # ---------------------------------------------------------------
        def dma_split(dst, src_ap, out_is_dram=False):
            # split by partition quarter so 4 DMA queues work in parallel;
            # alternate issuing engine: descriptor issue costs ~600ns each,
            # so two queues (Sync + idle GpSimd) double the issue rate
            for qi, r in enumerate(range(0, 128, 32)):
                eng = nc.gpsimd if qi % 2 else nc.sync
                eng.dma_start(out=dst[r : r + 32, :], in_=src_ap[r : r + 32, :])

        def emit_stream_dmas(j):
            w = EC * TJ
            dma_split(qs_j[j][:], qTt[:, j * w : (j + 1) * w])
            dma_split(ks_j[j][:], kTt[:, j * w : (j + 1) * w])
            dma_split(vs_j[j][:], vTt[:, j * w : (j + 1) * w])

        # explicit per-j stream tile handles (2 buffer sets, alternating);
        # q/k as fp8 chunk-pair tiles, v as bf16 per-chunk tiles
        qs_j, ks_j, vs_j = {}, {}, {}

        def alloc_stream_tiles(j):
            qs_j[j] = streams.tile([128, EC * TJ], FP8, tag="qs", name=f"qs_{j}", bufs=2)
            ks_j[j] = streams.tile([128, EC * TJ], FP8, tag="ks", name=f"ks_{j}", bufs=2)
            vs_j[j] = streams.tile([128, EC * TJ], BF16, tag="vs", name=f"vs_{j}", bufs=2)

        # ---------------------------------------------------------------
        # Projection chain closures
        # ---------------------------------------------------------------
        DR = mybir.MatmulPerfMode.DoubleRow

        def _dr_mm(ps, w_sb, s_tile, p, c, start, stop):
            pc = slice(p * 128, (p + 1) * 128)
            lhsT = w_sb[:, 2 * c * DC : (2 * c + 2) * DC].rearrange(
                "r (k m) -> r k m", k=2)[:, :, pc]
            rhs = s_tile[:, 2 * c * TJ : (2 * c + 2) * TJ].rearrange(
                "r (k n) -> r k n", k=2)
            nc.tensor.matmul(ps[:], lhsT, rhs, start=start, stop=stop,
                             perf_mode=DR)

        def push_qk_chain(p, j):
            """xq and xk chains for (p, j): DoubleRow fp8, 2 closures each."""
            st_q = {}

            def q1():
                ps = psA.tile([128, TJ], F32, tag="mm512", bufs=2, name=f"xqp{p}_{j}")
                st_q['ps'] = ps
                for c in range(2):
                    _dr_mm(ps, wq_sb, qs_j[j], p, c, c == 0, False)

            def q2():
                ps = st_q['ps']
                for c in range(2, EC // 2):
                    _dr_mm(ps, wq_sb, qs_j[j], p, c, False, c == EC // 2 - 1)
                nc.vector.tensor_copy(xq_t[p][j][:], ps[:])

            st_k = {}

            def k1():
                ps = psA.tile([128, TJ], F32, tag="mm512", bufs=2, name=f"xkp{p}_{j}")
                st_k['ps'] = ps
                for c in range(2):
                    _dr_mm(ps, wk_sb, ks_j[j], p, c, c == 0, False)

            def k2():
                ps = st_k['ps']
                for c in range(2, EC // 2):
                    _dr_mm(ps, wk_sb, ks_j[j], p, c, False, c == EC // 2 - 1)
                nc.vector.tensor_copy(xk_t[p][j][:], ps[:])

            push(1, q1)
            push(1, q2, key=("xq", p, j))
            push(1, k1)
            push(1, k2, key=("xk", p, j))

        def push_xv_chain(loc, j):
            si = 4 * j + loc
            st = {}

            def _vmm(ps, e, start, stop):
                lhsT = vs_j[j][:, e * TJ + loc * 128 : e * TJ + (loc + 1) * 128]
                rhs = wv_sb[:, e * DC : (e + 1) * DC]
                nc.tensor.matmul(ps[:], lhsT, rhs, start=start, stop=stop)

            def v1():
                ps = psA.tile([128, DC], F32, tag="mm512", bufs=2, name=f"xvp{si}")
                st['ps'] = ps
                for e in range(4):
                    _vmm(ps, e, e == 0, False)

            def v2():
                ps = st['ps']
                for e in range(4, EC):
                    _vmm(ps, e, False, e == EC - 1)
                nc.vector.tensor_copy(
                    xv_sb[si][:].rearrange("p (h x) -> p h x", x=65)[:, :, 0:64],
                    ps[:].rearrange("p (h d) -> p h d", h=HPC),
                )

            push(2, v1)
            push(2, v2, key=("xv", si))

        def push_proj(j, first_p_inline=False):
            """All projection chains for t-tile j, p0's q/k first."""
            order = []
            if not first_p_inline:
                order.append(("qk", 0))
            order += [("xv", loc) for loc in range(4)]
            order += [("qk", p) for p in range(1, NP)]
            for kind, a in order:
                if kind == "qk":
                    push_qk_chain(a, j)
                else:
                    push_xv_chain(a, j)

        # ---------------------------------------------------------------
        # Softmax tail + output projection closures
        # ---------------------------------------------------------------
        pending_tail = []    # (cost, fn, key) released at p==1 of next att(j)

        def pend(cost, fn, key=None):
            pending_tail.append((cost, fn, key))

        def release_pending():
            for cost, fn, key in pending_tail:
                push(cost, fn, key=key)
            pending_tail.clear()

        def push_tail(j, rcat_sb, osb_sb, osc_sb):
            def recip():
                # 1/r as exp(-ln r) on ACT: exp+ln share one table set; the
                # DVE reciprocal is lane-starved on the free dim (3.3us).
                rrs = []
                for t in range(2):
                    rln = work.tile([128, TJ], F32, tag=f"rln{t}", bufs=2,
                                    name=f"rln{t}_{j}")
                    nc.scalar.activation(rln[:], rcat_sb[t][:], LN, scale=1.0)
                    rr = work.tile([128, TJ], BF16, tag=f"rrc{t}", bufs=2,
                                   name=f"rrc{t}_{j}")
                    nc.scalar.activation(rr[:], rln[:], EXP, scale=-1.0)
                    rrs.append(rr)
                tail_state[j] = rrs

            pend(1, recip)

            for p in range(NP):
                def mk(p):
                    def rbosc():
                        rr = tail_state[j][p // 2]
                        rb_ps = psA.tile([128, TJ], F32, tag="mm512", bufs=2,
                                         name=f"rb_{p}_{j}")
                        nc.tensor.matmul(
                            rb_ps[:],
                            selexp_sb[:, (p % 2) * 128 : (p % 2 + 1) * 128],
                            rr[:], start=True, stop=True,
                        )
                        nc.vector.tensor_mul(osc_sb[p][:], osb_sb[p][:], rb_ps[:])
                    return rbosc
                pend(1, mk(p), key=("osc", p, j))

        def push_y(j, osc_sb):
            for m in range(EC):
                def mk(m):
                    def ychunk():
                        y_ps = psA.tile([128, TJ], F32, tag="mm512", bufs=2,
                                        name=f"y_{m}_{j}")
                        for p in range(NP):
                            nc.tensor.matmul(
                                y_ps[:], wp_sb[p][:, m * 128 : (m + 1) * 128],
                                osc_sb[p][:],
                                start=(p == 0), stop=(p == NP - 1),
                            )
                        y_sb = work.tile([128, TJ], F32, tag="y", bufs=2,
                                         name=f"ysb_{m}_{j}")
                        nc.vector.tensor_copy(y_sb[:], y_ps[:])
                        r0 = (j * EC + m) * 128
                        dma_split(yTt[r0 : r0 + 128, :], y_sb[:], out_is_dram=True)
                    return ychunk
                pend(2, mk(m))

        tail_state = {}

        # ---------------------------------------------------------------
        # Attention cell for (p, j): ACT-paced i-loop with filler pumping
        # ---------------------------------------------------------------
        def emit_attention(p, j):
            jt = slice(j * TJ, (j + 1) * TJ)
            ivals = []
            for i in range(NSI):
                types = [btab[i][4 * j + bl] for bl in range(4)]
                if all(t == "skip" for t in types):
                    continue
                ivals.append((i, types))
            n_i = len(ivals)

            require(("xq", p, j))

            o_ps = [
                psB.tile([65, TJ], F32, tag=f"ops{hh}", name=f"ops{hh}_{p}_{j}", bufs=1)
                for hh in range(2)
            ]
            touched = [[False] * 4, [False] * 4]
            sts = [None] * n_i     # (st_tile, u_tile, c0)

            def emit_pair(k):
                i, types = ivals[k]
                c0 = next(bl for bl in range(4) if types[bl] != "skip")
                require(("xk", p, i // 4))
                st = psA.tile([128, 2 * TJ], F32, tag="st", bufs=2)
                for hh in range(2):
                    hr = slice(hh * 64, (hh + 1) * 64)
                    nc.tensor.matmul(
                        st[:, hh * TJ + c0 * 128 : (hh + 1) * TJ],
                        xk_t[p][i // 4][hr, (i % 4) * 128 : (i % 4 + 1) * 128],
                        xq_t[p][j][hr, c0 * 128 : TJ],
                        start=True, stop=True,
                    )
                sts[k] = (st, None, c0)

            def emit_exp(k):
                st, _, c0 = sts[k]
                u = work.tile([128, 2 * TJ], BF16, tag="u", bufs=4)
                nc.scalar.activation(
                    u[:].rearrange("p (g c) -> p g c", g=2)[:, :, c0 * 128 : TJ],
                    st[:].rearrange("p (g c) -> p g c", g=2)[:, :, c0 * 128 : TJ],
                    EXP, scale=1.0 / (32.0 * 256.0),  # wq,wk pre-scaled x16 each
                )
                sts[k] = (st, u, c0)

            def emit_av(k):
                i, types = ivals[k]
                _, u, c0 = sts[k]
                require(("xv", i))
                for hh in range(2):
                    h = 2 * p + hh
                    uo = hh * TJ
                    runs = []  # (bl0, bl1, src_ap)
                    bl = c0
                    while bl < 4:
                        if types[bl] == "dense":
                            b2 = bl
                            while b2 + 1 < 4 and types[b2 + 1] == "dense":
                                b2 += 1
                            runs.append((bl, b2 + 1,
                                         u[:, uo + bl * 128 : uo + (b2 + 1) * 128]))
                            bl = b2 + 1
                        elif types[bl] == "skip":
                            bl += 1
                        else:
                            mt = work.tile([128, 128], BF16, tag="mfix", bufs=4)
                            nc.vector.tensor_mul(
                                mt[:], u[:, uo + bl * 128 : uo + (bl + 1) * 128],
                                pat_sb[types[bl]][:],
                            )
                            runs.append((bl, bl + 1, mt[:]))
                            bl += 1
                    lhs_v = xv_sb[i][:, h * 65 : h * 65 + 65]
                    for ri, (b0, b1, src) in enumerate(runs):
                        first = all(not touched[hh][b] for b in range(b0, b1))
                        assert first == any(
                            not touched[hh][b] for b in range(b0, b1)
                        ), "mask blocks: mixed touch state inside a run"
                        last = (k == n_i - 1) and (ri == len(runs) - 1)
                        nc.tensor.matmul(
                            o_ps[hh][:, b0 * 128 : b1 * 128],
                            lhs_v, src,
                            start=first, stop=last,
                            skip_group_check=True,
                        )
                        for b in range(b0, b1):
                            touched[hh][b] = True
                sts[k] = None

            # pipelined i-loop, two steps per "mode era" to halve the
            # 64<->128 row-tiling mode switches: [pair pair] [exp exp]
            # [fillers AV AV]
            emit_pair(0)
            if n_i > 1:
                emit_pair(1)
            emit_exp(0)
            if n_i > 1:
                emit_exp(1)
            k = 0
            while k < n_i:
                k2 = min(k + 2, n_i)
                for kk in range(k + 2, min(k + 4, n_i)):
                    emit_pair(kk)
                for kk in range(k + 2, min(k + 4, n_i)):
                    emit_exp(kk)
                # AV(k) right after the pairs: the 64->128 mode switch lands
                # on its cheap 65-col LDWEIGHTS, and exp(k) finished last era
                emit_av(k)
                if p == 0 and k >= 6 and pending_tail:
                    release_pending()
                pump(k2 - k)
                for kk in range(k + 1, k2):
                    emit_av(kk)
                k = k2

            require(("osc", p, j - 2))  # osb buffer of j-2 must be fully consumed
            # stage row sums into rcat tile p//2, 32-aligned partition
            # (32*(h%4)) -- legal DVE partition bases, no DMA gather needed
            for hh in range(2):
                h = 2 * p + hh
                r0 = 32 * (h % 4)
                nc.vector.tensor_copy(
                    rcat_cur[0][p // 2][r0 : r0 + 1, :], o_ps[hh][64:65, :])
                nc.vector.tensor_copy(
                    osb_cur[0][p][hh * 64 : (hh + 1) * 64, :], o_ps[hh][0:64, :]
                )

        rcat_cur = [None]
        osb_cur = [None]

        # ---------------------------------------------------------------
        # Prologue
        # ---------------------------------------------------------------
        # warm the ACT exp table while DMAs run
        nc.vector.memset(dummy_sb[:], 0.0)
        nc.scalar.activation(dummy_sb[:, 0:1], dummy_sb[:, 1:2], EXP, scale=1.0)
        for i in range(NSI):
            nc.vector.memset(
                xv_sb[i][:].rearrange("p (h x) -> p h x", x=65)[:, :, 64:65], 1.0
            )

        alloc_stream_tiles(0)
        # first-needed DMAs first: wq+qs(0)+wk+ks(0), small consts, wv+vs(0);
        # j=1 streams and wp only after the j=0-critical loads
        w = EC * TJ
        dma_split(wq_sb[:], wq[:])
        dma_split(qs_j[0][:], qTt[:, 0:w])
        dma_split(wk_sb[:], wk[:])
        dma_split(ks_j[0][:], kTt[:, 0:w])
        for u in range(n_pat):
            nc.sync.dma_start(out=pat_sb[u][:], in_=pat[u * 128 : (u + 1) * 128, :])
        nc.sync.dma_start(out=selexp_sb[:], in_=selexp[:])
        dma_split(wv_sb[:], wv[:])
        dma_split(vs_j[0][:], vTt[:, 0:w])

        # PE warm-up on scratch data while DMAs land: pushes the HAM clock
        # gate to 8/8 before the first real matmul (garbage results, unread)
        warm_sb = consts.tile([128, TJ], BF16, tag="warm", name="warm", bufs=1)
        nc.vector.memset(warm_sb[:], 1.0)
        for wi in range(36):
            wps = psA.tile([128, TJ], F32, tag="mm512", bufs=2, name=f"warm{wi}")
            nc.tensor.matmul(wps[:], warm_sb[:, 0:128], warm_sb[:],
                             start=True, stop=True)

        # inline p0 projections for j=0, rest queued
        push_qk_chain(0, 0)
        require(("xk", 0, 0))
        push_proj(0, first_p_inline=True)

        # ---------------------------------------------------------------
        # Main loop
        # ---------------------------------------------------------------
        for j in range(NJ):
            if j == 0:
                pass  # proj(1) pushed mid-att(0), after its stream DMAs
            elif j + 1 < NJ:
                push_proj(j + 1)

            rcat_cur[0] = [
                work.tile([128, TJ], BF16, tag=f"rcat{t}", bufs=2, name=f"rcat{t}_{j}")
                for t in range(2)
            ]
            if j < 2:
                # unused rows stay 1.0 forever (ln->0, exp->1; selector zeroes
                # them), so only the first two buffer generations need memset
                for t in range(2):
                    nc.vector.memset(rcat_cur[0][t][:], 1.0)
            osb_cur[0] = [
                work.tile([128, TJ], BF16, tag=f"osb{p}", bufs=2, name=f"osb{p}_{j}")
                for p in range(NP)
            ]
            osc_sb = osc_sb_all[j % 2]
            rcat_sb = rcat_cur[0]
            osb_sb = osb_cur[0]

            for p in range(NP):
                if p == 1:
                    if j == 0:
                        # j=1 streams + wp deferred out of the startup window
                        alloc_stream_tiles(1)
                        emit_stream_dmas(1)
                        for pp in range(NP):
                            nc.sync.dma_start(
                                out=wp_sb[pp][:],
                                in_=wpT[pp * 128 : (pp + 1) * 128, :])
                        push_proj(1)
                    release_pending()
                emit_attention(p, j)

            # all proj(j) closures are drained by now (att(j) p=3 required
            # ("xk", 3, j)), so the buffer-recycling DMA is safe to emit
            if j + 2 < NJ:
                alloc_stream_tiles(j + 2)
                emit_stream_dmas(j + 2)

            push_tail(j, rcat_sb, osb_sb, osc_sb)
            push_y(j, osc_sb)

        # drain everything left (tail + y of the last tiles)
        release_pending()
        while fillers:
            _run_one()

    _split_multi_waits(nc)
    return nc


# one-hot row selector: col block pp (=p%2) picks rows 32*(2*pp) and
# 32*(2*pp+1) for output partitions 0-63 / 64-127
_SELEXP = np.zeros((128, 256), NPBF16)
for _pp in range(2):
    _SELEXP[32 * (2 * _pp), _pp * 128 : _pp * 128 + 64] = 1.0
    _SELEXP[32 * (2 * _pp + 1), _pp * 128 + 64 : _pp * 128 + 128] = 1.0

_CACHE = {}


def _get_program(mask):
    key = np.asarray(mask, dtype=bool).tobytes()
    prog = _CACHE.get(key)
    if prog is None:
        _install_patches()
        btab, patterns = _classify_mask(mask)
        nc = _build(btab, len(patterns))
        prog = (nc, patterns)
        _CACHE[key] = prog
    return prog


def _prepare(k, q, v, mask, Wk, Wq, Wv, Wp):
    """Build (cached) the SPMD program and the 8 per-core input maps."""
    k = np.asarray(k, np.float32)
    q = np.asarray(q, np.float32)
    v = np.asarray(v, np.float32)
    Wk = np.asarray(Wk, np.float32)
    Wq = np.asarray(Wq, np.float32)
    Wv = np.asarray(Wv, np.float32)
    Wp = np.asarray(Wp, np.float32)

    nc, patterns = _get_program(mask)
    patflat = np.ascontiguousarray(patterns.reshape(-1, 128))

    def tr_tiled(x, dt):
        # [T, E] f32 -> partition-major [128, NJ*EC*TJ]: partition p holds
        # x^T[e*128+p, j*TJ+c] at col ((j*EC+e)*TJ + c) -> contiguous 1-packet
        # per-partition DMAs
        xt = np.ascontiguousarray(x.astype(dt).T)          # [E, T]
        xt = xt.reshape(EC, 128, NJ, TJ).transpose(1, 2, 0, 3)  # [128, NJ, EC, TJ]
        return np.ascontiguousarray(xt.reshape(128, NJ * EC * TJ))

    def wcat(W, half, dt, scale=1.0):
        # [H, E, D] -> partition-major [128, EC*DC] (e-chunk c at cols c*DC)
        w = (W[half * HPC : (half + 1) * HPC].transpose(1, 0, 2)
             .reshape(E, DC) * scale).astype(dt)
        w = w.reshape(EC, 128, DC).transpose(1, 0, 2)
        return np.ascontiguousarray(w.reshape(128, EC * DC))

    in_maps = []
    for c in range(8):
        b, half = divmod(c, 2)
        off = half * DC
        in_maps.append(
            {
                "qTt": tr_tiled(q[b], NPFP8),
                "kTt": tr_tiled(k[b], NPFP8),
                "vTt": tr_tiled(v[b], NPBF16),
                # wq/wk pre-scaled x16 for fp8 range; exp scale divides it out
                "wq": wcat(Wq, half, NPFP8, 16.0),
                "wk": wcat(Wk, half, NPFP8, 16.0),
                "wv": wcat(Wv, half, NPBF16),
                "wpT": np.ascontiguousarray(Wp[:, off : off + DC].T).astype(NPBF16),
                "pat": patflat,
                "selexp": _SELEXP,
            }
        )
    return nc, in_maps


def kernel(k, q, v, mask, Wk, Wq, Wv, Wp, bp):
    bp = np.asarray(bp, np.float32)
    nc, in_maps = _prepare(k, q, v, mask, Wk, Wq, Wv, Wp)
    res = run_bass_kernel_spmd(nc, in_maps, list(range(8)))
    out = np.empty((B, T, E), np.float32)
    for b in range(B):
        yt = res.results[2 * b]["yTt"] + res.results[2 * b + 1]["yTt"]
        # [NJ*EC*128, TJ] -> [E, T]
        yt = yt.reshape(NJ, EC, 128, TJ).transpose(1, 2, 0, 3).reshape(E, T)
        out[b] = yt.T + bp[None, :]
    return out


# revision 34
# speedup vs baseline: 1.0442x; 1.0442x over previous
"""Multi-head causal attention (B=4, T=2048, E=1024, H=16, D=64) on 8 trn2
NeuronCores via Bass/Tile.

Sharding: core c handles batch b = c//2 and heads [half*8, half*8+8), half =
c%2. Each core computes its 8 heads' attention and a partial output
projection; the host sums the two half partials per batch, transposes, and
adds the bias.

On-device layout is "transposed": activations are [feature, token] so every
matmul contracts over the partition dim. Softmax denominators come from a
ones-column appended to the stationary V operand (M=65 matmuls); masking is
applied block-wise (128x128) with patterns derived from the actual mask input
at build time. No max-subtraction is needed: scores are ~N(0, 0.083^2).

This version software-pipelines the whole kernel: the attention i-loop is
ACT(exp)-paced, so projection matmuls for the next t-tile, output-projection
matmuls for the previous t-tile, and softmax-tail work are injected as
"filler" closures between attention steps to keep the PE busy. Inputs are
pre-tiled host-side so every DMA moves one contiguous 128KB block.
"""
import numpy as np
import ml_dtypes
from collections import deque
from contextlib import ExitStack

import concourse.bass as bass
import concourse.mybir as mybir
import concourse.tile as tile
from concourse.bass_utils import run_bass_kernel_spmd
from concourse.vector_clock import ScopedClock

BF16 = mybir.dt.bfloat16
F32 = mybir.dt.float32
NPBF16 = ml_dtypes.bfloat16
NPFP8 = ml_dtypes.float8_e4m3fn

B, T, E, H, D = 4, 2048, 1024, 16, 64
HPC = 8            # heads per core
DC = HPC * D       # 512: stacked head dim per core
TJ = 512           # t tile (matmul free dim)
NJ = T // TJ       # 4
SI = 128           # s tile (psum partition dim)
NSI = T // SI      # 16
EC = E // 128      # 8 e-chunks
NP = HPC // 2      # 4 head pairs

# ---------------------------------------------------------------------------
# Workarounds for this walrus build: at most ONE sync wait per instruction.
# ---------------------------------------------------------------------------
_PATCHED = False


def _patched_drain_and_barrier(self, tick_clock, wait_clock):
    drain_inst = self.nc.sync.drain(fusable=False)
    wait_clock.add_sem_waits(
        drain_inst.ins, ScopedClock({None: tick_clock.global_clock})
    )
    si = drain_inst.ins.sync_info
    if si is not None and len(si.on_wait) > 1:
        waits = list(si.on_wait)
        drain_inst.ins.sync_info = mybir.SyncInfo(
            on_wait=waits[:1], on_update=list(si.on_update)
        )
        for ofs in range(1, len(waits)):
            extra = self.nc.sync.drain(fusable=False)
            extra.ins.sync_info = mybir.SyncInfo(
                on_wait=waits[ofs : ofs + 1], on_update=[]
            )
    self.nc.all_engine_barrier()
    assert self.sems is not None
    popped = self.nc._tile_sem_poison_stack.pop()
    assert popped is self._sem_poison
    self.nc.clear_and_free_semaphores(list(self.sems.allocated().values()))
    self.nc.all_engine_barrier()


def _install_patches():
    global _PATCHED
    if _PATCHED:
        return
    tile.TileContext._drain_and_barrier = _patched_drain_and_barrier
    _PATCHED = True


def _make_carrier(nc, engine, wait):
    """Wait-only EventSemaphore on `engine` (cheap: ~70ns, no pipe flush)."""
    ev = mybir.InstEventSemaphore(name=f"W-{nc.next_id()}", ins=[], outs=[])
    ev.engine = engine
    ev.sync_info = mybir.SyncInfo(on_wait=[wait], on_update=[])
    return ev


_ENGINE_SEM = {
    "EngineType.PE": "PE",
    "EngineType.DVE": "DVE",
    "EngineType.Activation": "Activation",
    "EngineType.SP": "SP",
    "EngineType.Pool": "Pool",
}
# engines with in-order issue AND in-order completion for these inst types:
# a wait on the engine's own completion sem is redundant. Ldweights excluded
# (the PE reorder window pulls it ahead of in-flight matmuls).
_DROPPABLE = (
    "InstMatmult", "InstActivation", "InstTensorTensor", "InstTensorCopy",
    "InstTensorReduce", "InstMemset", "InstReciprocal", "InstDMACopy",
    "InstCopyPredicated", "InstTensorScalarPtr", "InstTensorScalar",
    "InstCast", "InstDveOp", "InstCustomDve",
)


def _split_multi_waits(nc):
    for bbw in list(nc.bb_map.values()):
        bb = bbw.bb
        insts = bb.instructions
        if not any(
            getattr(i, "sync_info", None) is not None and len(i.sync_info.on_wait) > 1
            for i in insts
        ):
            continue
        out = []
        for inst in insts:
            si = getattr(inst, "sync_info", None)
            waits = list(si.on_wait) if si is not None else []
            if len(waits) > 1:
                own = _ENGINE_SEM.get(str(inst.engine))
                tn = type(inst).__name__
                if own is not None and tn.startswith(_DROPPABLE):
                    waits = [
                        w for w in waits
                        if w.ant_name.rsplit("_", 1)[0] != own
                    ] or waits[-1:]
            if len(waits) > 1:
                for w in waits[:-1]:
                    out.append(_make_carrier(nc, inst.engine, w))
                waits = waits[-1:]
            if si is not None and list(si.on_wait) != waits:
                inst.sync_info = mybir.SyncInfo(
                    on_wait=waits, on_update=list(si.on_update)
                )
            out.append(inst)
        insts[:] = out


# ---------------------------------------------------------------------------
# Mask analysis (host side, 128x128 blocks).
# ---------------------------------------------------------------------------
def _classify_mask(mask):
    """mask: [T, T] bool, mask[t, s]=True means masked (score -> -inf).

    Returns (btab, patterns): btab[i][jj] in {'skip', 'dense', int u};
    patterns[u] is a [128,128] bf16 multiplier in [s, t] orientation."""
    nb = T // 128
    m = np.asarray(mask, dtype=bool)
    patterns = []
    index = {}
    btab = [[None] * nb for _ in range(nb)]
    for i in range(nb):          # s block
        for jj in range(nb):     # t block
            sub = m[jj * 128 : (jj + 1) * 128, i * 128 : (i + 1) * 128]  # [t, s]
            if sub.all():
                btab[i][jj] = "skip"
            elif not sub.any():
                btab[i][jj] = "dense"
            else:
                pat = (~sub).T.astype(NPBF16)  # [s, t] multiplier
                key = pat.tobytes()
                if key not in index:
                    index[key] = len(patterns)
                    patterns.append(pat)
                btab[i][jj] = index[key]
    if not patterns:
        patterns.append(np.ones((128, 128), NPBF16))
    return btab, np.stack(patterns)


# ---------------------------------------------------------------------------
# Kernel builder (SPMD program, identical on all 8 cores).
# ---------------------------------------------------------------------------
def _build(btab, n_pat):
    nc = bass.Bass()
    FP8 = mybir.dt.float8e4
    # pre-tiled streams: tile (j, e) is rows (j*EC+e)*128 .. +128, contiguous
    # q/k are fp8 (DoubleRow matmuls; errors attenuate through softmax)
    qTt = nc.declare_dram_parameter("qTt", [128, NJ * EC * TJ], FP8, isOutput=False)
    kTt = nc.declare_dram_parameter("kTt", [128, NJ * EC * TJ], FP8, isOutput=False)
    vTt = nc.declare_dram_parameter("vTt", [128, NJ * EC * TJ], BF16, isOutput=False)
    wq = nc.declare_dram_parameter("wq", [128, EC * DC], FP8, isOutput=False)
    wk = nc.declare_dram_parameter("wk", [128, EC * DC], FP8, isOutput=False)
    wv = nc.declare_dram_parameter("wv", [128, EC * DC], BF16, isOutput=False)
    wpT = nc.declare_dram_parameter("wpT", [DC, E], BF16, isOutput=False)
    pat = nc.declare_dram_parameter("pat", [n_pat * 128, 128], BF16, isOutput=False)
    selexp = nc.declare_dram_parameter("selexp", [128, 256], BF16, isOutput=False)
    # pre-tiled output: tile (j, m) is rows (j*EC+m)*128 .. +128
    yTt = nc.declare_dram_parameter("yTt", [NJ * EC * 128, TJ], BF16, isOutput=True)

    EXP = mybir.ActivationFunctionType.Exp
    LN = mybir.ActivationFunctionType.Ln

    with ExitStack() as ctx:
        tc = ctx.enter_context(tile.TileContext(nc))
        # SBUF pools
        consts = ctx.enter_context(tc.tile_pool(name="consts", bufs=1))
        streams = ctx.enter_context(tc.tile_pool(name="streams", bufs=1))
        acts = ctx.enter_context(tc.tile_pool(name="acts", bufs=1))
        work = ctx.enter_context(tc.tile_pool(name="work", bufs=1))
        # PSUM pools
        psA = ctx.enter_context(tc.tile_pool(name="psA", bufs=1, space="PSUM"))
        psB = ctx.enter_context(tc.tile_pool(name="psB", bufs=1, space="PSUM"))

        # ---- constants ----
        # weights as single partition-major tiles; e-chunk c lives at
        # cols [c*DC, (c+1)*DC)
        wq_sb = consts.tile([128, EC * DC], FP8, tag="wq", name="wq", bufs=1)
        wk_sb = consts.tile([128, EC * DC], FP8, tag="wk", name="wk", bufs=1)
        wv_sb = consts.tile([128, EC * DC], BF16, tag="wv", name="wv", bufs=1)
        wp_sb = [consts.tile([128, E], BF16, tag=f"wp{p}", name=f"wp{p}", bufs=1) for p in range(NP)]
        pat_sb = [consts.tile([128, 128], BF16, tag=f"pat{u}", name=f"pat{u}", bufs=1) for u in range(n_pat)]
        selexp_sb = consts.tile([128, 256], BF16, tag="selexp", name="selexp", bufs=1)
        dummy_sb = consts.tile([1, 2], BF16, tag="dummy", name="dummy", bufs=1)

        # ---- persistent activations (per (p, j) tiles so pipelined writes
        # to tile j+1 never alias reads of tile j) ----
        xq_t = [[acts.tile([128, TJ], BF16, tag=f"xq{p}_{j}", name=f"xq{p}_{j}", bufs=1)
                 for j in range(NJ)] for p in range(NP)]
        xk_t = [[acts.tile([128, TJ], BF16, tag=f"xk{p}_{j}", name=f"xk{p}_{j}", bufs=1)
                 for j in range(NJ)] for p in range(NP)]
        # xv tiles: per s-tile, heads laid out as 8 x (64 cols xv | 1 col ones)
        xv_sb = [acts.tile([128, HPC * 65], BF16, tag=f"xv{i}", name=f"xv{i}", bufs=1) for i in range(NSI)]
        osc_sb_all = [
            [acts.tile([128, TJ], BF16, tag=f"osc{p}_{jj}", name=f"osc{p}_{jj}", bufs=1)
             for p in range(NP)]
            for jj in range(2)
        ]

        # ---------------------------------------------------------------
        # Filler queue: closures emitting ~1-2 PE matmuls (+ their DVE/DMA
        # tails). `require(key)` force-drains through a named closure.
        # ---------------------------------------------------------------
        fillers = deque()        # entries: (cost, fn)
        ready_idx = {}           # key -> push counter of last closure for key
        drained = [0]            # count of executed closures
        pushed = [0]
        debt = [0.0]

        def push(cost, fn, key=None):
            fillers.append((cost, fn))
            pushed[0] += 1
            if key is not None:
                ready_idx[key] = pushed[0]

        def _run_one():
            cost, fn = fillers.popleft()
            fn()
            drained[0] += 1
            return cost

        def pump(units):
            debt[0] += units
            while fillers and debt[0] >= fillers[0][0]:
                debt[0] -= _run_one()

        def require(key):
            idx = ready_idx.get(key, 0)
            while drained[0] < idx:
                _run_one()

        # ---------------------------------------------------------------
        # DMA emission
        # ---------------------------------------------------------------
        def dma_split(dst, src_ap, out_is_dram=False):
            # split by partition quarter so 4 DMA queues work in parallel;
            # alternate issuing engine: descriptor issue costs ~600ns each,
            # so two queues (Sync + idle GpSimd) double the issue rate
            for qi, r in enumerate(range(0, 128, 32)):
                eng = nc.gpsimd if qi % 2 else nc.sync
                eng.dma_start(out=dst[r : r + 32, :], in_=src_ap[r : r + 32, :])

        def emit_stream_dmas(j):
            w = EC * TJ
            dma_split(qs_j[j][:], qTt[:, j * w : (j + 1) * w])
            dma_split(ks_j[j][:], kTt[:, j * w : (j + 1) * w])
            dma_split(vs_j[j][:], vTt[:, j * w : (j + 1) * w])

        # explicit per-j stream tile handles (2 buffer sets, alternating);
        # q/k as fp8 chunk-pair tiles, v as bf16 per-chunk tiles
        qs_j, ks_j, vs_j = {}, {}, {}

        def alloc_stream_tiles(j):
            qs_j[j] = streams.tile([128, EC * TJ], FP8, tag="qs", name=f"qs_{j}", bufs=2)
            ks_j[j] = streams.tile([128, EC * TJ], FP8, tag="ks", name=f"ks_{j}", bufs=2)
            vs_j[j] = streams.tile([128, EC * TJ], BF16, tag="vs", name=f"vs_{j}", bufs=2)

        # ---------------------------------------------------------------
        # Projection chain closures
        # ---------------------------------------------------------------
        DR = mybir.MatmulPerfMode.DoubleRow

        def _dr_mm(ps, w_sb, s_tile, p, c, start, stop):
            pc = slice(p * 128, (p + 1) * 128)
            lhsT = w_sb[:, 2 * c * DC : (2 * c + 2) * DC].rearrange(
                "r (k m) -> r k m", k=2)[:, :, pc]
            rhs = s_tile[:, 2 * c * TJ : (2 * c + 2) * TJ].rearrange(
                "r (k n) -> r k n", k=2)
            nc.tensor.matmul(ps[:], lhsT, rhs, start=start, stop=stop,
                             perf_mode=DR)

        def push_qk_chain(p, j):
            """xq and xk chains for (p, j): DoubleRow fp8, 2 closures each."""
            st_q = {}

            def q1():
                ps = psA.tile([128, TJ], F32, tag="mm512", bufs=2, name=f"xqp{p}_{j}")
                st_q['ps'] = ps
                for c in range(2):
                    _dr_mm(ps, wq_sb, qs_j[j], p, c, c == 0, False)

            def q2():
                ps = st_q['ps']
                for c in range(2, EC // 2):
                    _dr_mm(ps, wq_sb, qs_j[j], p, c, False, c == EC // 2 - 1)
                nc.vector.tensor_copy(xq_t[p][j][:], ps[:])

            st_k = {}

            def k1():
                ps = psA.tile([128, TJ], F32, tag="mm512", bufs=2, name=f"xkp{p}_{j}")
                st_k['ps'] = ps
                for c in range(2):
                    _dr_mm(ps, wk_sb, ks_j[j], p, c, c == 0, False)

            def k2():
                ps = st_k['ps']
                for c in range(2, EC // 2):
                    _dr_mm(ps, wk_sb, ks_j[j], p, c, False, c == EC // 2 - 1)
                nc.vector.tensor_copy(xk_t[p][j][:], ps[:])

            push(1, q1)
            push(1, q2, key=("xq", p, j))
            push(1, k1)
            push(1, k2, key=("xk", p, j))

        def push_xv_chain(loc, j):
            si = 4 * j + loc
            st = {}

            def _vmm(ps, e, start, stop):
                lhsT = vs_j[j][:, e * TJ + loc * 128 : e * TJ + (loc + 1) * 128]
                rhs = wv_sb[:, e * DC : (e + 1) * DC]
                nc.tensor.matmul(ps[:], lhsT, rhs, start=start, stop=stop)

            def v1():
                ps = psA.tile([128, DC], F32, tag="mm512", bufs=2, name=f"xvp{si}")
                st['ps'] = ps
                for e in range(4):
                    _vmm(ps, e, e == 0, False)

            def v2():
                ps = st['ps']
                for e in range(4, EC):
                    _vmm(ps, e, False, e == EC - 1)
                nc.vector.tensor_copy(
                    xv_sb[si][:].rearrange("p (h x) -> p h x", x=65)[:, :, 0:64],
                    ps[:].rearrange("p (h d) -> p h d", h=HPC),
                )

            push(2, v1)
            push(2, v2, key=("xv", si))

        def push_proj(j, first_p_inline=False):
            """All projection chains for t-tile j, p0's q/k first."""
            order = []
            if not first_p_inline:
                order.append(("qk", 0))
            order += [("xv", loc) for loc in range(4)]
            order += [("qk", p) for p in range(1, NP)]
            for kind, a in order:
                if kind == "qk":
                    push_qk_chain(a, j)
                else:
                    push_xv_chain(a, j)

        # ---------------------------------------------------------------
        # Softmax tail + output projection closures
        # ---------------------------------------------------------------
        pending_tail = []    # (cost, fn, key) released at p==1 of next att(j)

        def pend(cost, fn, key=None):
            pending_tail.append((cost, fn, key))

        def release_pending(keep=0):
            take = pending_tail[: len(pending_tail) - keep if keep else None]
            rest = pending_tail[len(take):]
            for cost, fn, key in take:
                push(cost, fn, key=key)
            pending_tail[:] = rest

        def push_tail(j, rcat_sb, osb_sb, osc_sb):
            def recip():
                # 1/r as exp(-ln r) on ACT: exp+ln share one table set; the
                # DVE reciprocal is lane-starved on the free dim (3.3us).
                rrs = []
                for t in range(2):
                    rln = work.tile([128, TJ], F32, tag=f"rln{t}", bufs=2,
                                    name=f"rln{t}_{j}")
                    nc.scalar.activation(rln[:], rcat_sb[t][:], LN, scale=1.0)
                    rr = work.tile([128, TJ], BF16, tag=f"rrc{t}", bufs=2,
                                   name=f"rrc{t}_{j}")
                    nc.scalar.activation(rr[:], rln[:], EXP, scale=-1.0)
                    rrs.append(rr)
                tail_state[j] = rrs

            pend(4, recip)

            for p in range(NP):
                def mk(p):
                    def rbosc():
                        rr = tail_state[j][p // 2]
                        rb_ps = psA.tile([128, TJ], F32, tag="mm512", bufs=2,
                                         name=f"rb_{p}_{j}")
                        nc.tensor.matmul(
                            rb_ps[:],
                            selexp_sb[:, (p % 2) * 128 : (p % 2 + 1) * 128],
                            rr[:], start=True, stop=True,
                        )
                        nc.vector.tensor_mul(osc_sb[p][:], osb_sb[p][:], rb_ps[:])
                    return rbosc
                pend(1, mk(p), key=("osc", p, j))

        def push_y(j, osc_sb):
            for m in range(EC):
                def mk(m):
                    def ychunk():
                        y_ps = psA.tile([128, TJ], F32, tag="mm512", bufs=2,
                                        name=f"y_{m}_{j}")
                        for p in range(NP):
                            nc.tensor.matmul(
                                y_ps[:], wp_sb[p][:, m * 128 : (m + 1) * 128],
                                osc_sb[p][:],
                                start=(p == 0), stop=(p == NP - 1),
                            )
                        y_sb = work.tile([128, TJ], BF16, tag="y", bufs=2,
                                         name=f"ysb_{m}_{j}")
                        nc.vector.tensor_copy(y_sb[:], y_ps[:])
                        r0 = (j * EC + m) * 128
                        nc.sync.dma_start(out=yTt[r0 : r0 + 128, :], in_=y_sb[:])
                    return ychunk
                pend(2, mk(m))

        tail_state = {}

        # ---------------------------------------------------------------
        # Attention cell for (p, j): ACT-paced i-loop with filler pumping
        # ---------------------------------------------------------------
        def emit_attention(p, j):
            jt = slice(j * TJ, (j + 1) * TJ)
            ivals = []
            for i in range(NSI):
                types = [btab[i][4 * j + bl] for bl in range(4)]
                if all(t == "skip" for t in types):
                    continue
                ivals.append((i, types))
            n_i = len(ivals)

            require(("xq", p, j))

            o_ps = [
                psB.tile([65, TJ], F32, tag=f"ops{hh}", name=f"ops{hh}_{p}_{j}", bufs=1)
                for hh in range(2)
            ]
            touched = [[False] * 4, [False] * 4]
            sts = [None] * n_i     # (st_tile, u_tile, c0)

            def emit_pair(k):
                i, types = ivals[k]
                c0 = next(bl for bl in range(4) if types[bl] != "skip")
                require(("xk", p, i // 4))
                st = psA.tile([128, 2 * TJ], F32, tag="st", bufs=2)
                for hh in range(2):
                    hr = slice(hh * 64, (hh + 1) * 64)
                    nc.tensor.matmul(
                        st[:, hh * TJ + c0 * 128 : (hh + 1) * TJ],
                        xk_t[p][i // 4][hr, (i % 4) * 128 : (i % 4 + 1) * 128],
                        xq_t[p][j][hr, c0 * 128 : TJ],
                        start=True, stop=True,
                    )
                sts[k] = (st, None, c0)

            def emit_exp(k):
                st, _, c0 = sts[k]
                u = work.tile([128, 2 * TJ], BF16, tag="u", bufs=4)
                nc.scalar.activation(
                    u[:].rearrange("p (g c) -> p g c", g=2)[:, :, c0 * 128 : TJ],
                    st[:].rearrange("p (g c) -> p g c", g=2)[:, :, c0 * 128 : TJ],
                    EXP, scale=1.0 / (32.0 * 256.0),  # wq,wk pre-scaled x16 each
                )
                sts[k] = (st, u, c0)

            def emit_av(k):
                i, types = ivals[k]
                _, u, c0 = sts[k]
                require(("xv", i))
                for hh in range(2):
                    h = 2 * p + hh
                    uo = hh * TJ
                    runs = []  # (bl0, bl1, src_ap)
                    bl = c0
                    while bl < 4:
                        if types[bl] == "dense":
                            b2 = bl
                            while b2 + 1 < 4 and types[b2 + 1] == "dense":
                                b2 += 1
                            runs.append((bl, b2 + 1,
                                         u[:, uo + bl * 128 : uo + (b2 + 1) * 128]))
                            bl = b2 + 1
                        elif types[bl] == "skip":
                            bl += 1
                        else:
                            mt = work.tile([128, 128], BF16, tag="mfix", bufs=4)
                            nc.vector.tensor_mul(
                                mt[:], u[:, uo + bl * 128 : uo + (bl + 1) * 128],
                                pat_sb[types[bl]][:],
                            )
                            runs.append((bl, bl + 1, mt[:]))
                            bl += 1
                    lhs_v = xv_sb[i][:, h * 65 : h * 65 + 65]
                    for ri, (b0, b1, src) in enumerate(runs):
                        first = all(not touched[hh][b] for b in range(b0, b1))
                        assert first == any(
                            not touched[hh][b] for b in range(b0, b1)
                        ), "mask blocks: mixed touch state inside a run"
                        last = (k == n_i - 1) and (ri == len(runs) - 1)
                        nc.tensor.matmul(
                            o_ps[hh][:, b0 * 128 : b1 * 128],
                            lhs_v, src,
                            start=first, stop=last,
                            skip_group_check=True,
                        )
                        for b in range(b0, b1):
                            touched[hh][b] = True
                sts[k] = None

            # pipelined i-loop, two steps per "mode era" to halve the
            # 64<->128 row-tiling mode switches: [pair pair] [exp exp]
            # [fillers AV AV]
            emit_pair(0)
            if n_i > 1:
                emit_pair(1)
            emit_exp(0)
            if n_i > 1:
                emit_exp(1)
            k = 0
            while k < n_i:
                k2 = min(k + 2, n_i)
                for kk in range(k + 2, min(k + 4, n_i)):
                    emit_pair(kk)
                for kk in range(k + 2, min(k + 4, n_i)):
                    emit_exp(kk)
                # AV(k) right after the pairs: the 64->128 mode switch lands
                # on its cheap 65-col LDWEIGHTS, and exp(k) finished last era
                emit_av(k)
                if p == 0 and k >= 6 and pending_tail:
                    release_pending(keep=6)
                if p == 2 and k == 0 and pending_tail:
                    release_pending()
                pump(k2 - k)
                for kk in range(k + 1, k2):
                    emit_av(kk)
                k = k2

            require(("osc", p, j - 2))  # osb buffer of j-2 must be fully consumed
            # stage row sums into rcat tile p//2, 32-aligned partition
            # (32*(h%4)) -- legal DVE partition bases, no DMA gather needed
            for hh in range(2):
                h = 2 * p + hh
                r0 = 32 * (h % 4)
                nc.vector.tensor_copy(
                    rcat_cur[0][p // 2][r0 : r0 + 1, :], o_ps[hh][64:65, :])
                nc.vector.tensor_copy(
                    osb_cur[0][p][hh * 64 : (hh + 1) * 64, :], o_ps[hh][0:64, :]
                )

        rcat_cur = [None]
        osb_cur = [None]

        # ---------------------------------------------------------------
        # Prologue
        # ---------------------------------------------------------------
        # warm the ACT exp table while DMAs run
        nc.vector.memset(dummy_sb[:], 0.0)
        nc.scalar.activation(dummy_sb[:, 0:1], dummy_sb[:, 1:2], EXP, scale=1.0)
        for i in range(NSI):
            nc.vector.memset(
                xv_sb[i][:].rearrange("p (h x) -> p h x", x=65)[:, :, 64:65], 1.0
            )

        alloc_stream_tiles(0)
        # first-needed DMAs first: wq+qs(0)+wk+ks(0), small consts, wv+vs(0);
        # j=1 streams and wp only after the j=0-critical loads
        w = EC * TJ
        dma_split(wq_sb[:], wq[:])
        dma_split(qs_j[0][:], qTt[:, 0:w])
        dma_split(wk_sb[:], wk[:])
        dma_split(ks_j[0][:], kTt[:, 0:w])
        for u in range(n_pat):
            nc.sync.dma_start(out=pat_sb[u][:], in_=pat[u * 128 : (u + 1) * 128, :])
        nc.sync.dma_start(out=selexp_sb[:], in_=selexp[:])
        dma_split(wv_sb[:], wv[:])
        dma_split(vs_j[0][:], vTt[:, 0:w])

        # PE warm-up on scratch data while DMAs land: pushes the HAM clock
        # gate to 8/8 before the first real matmul (garbage results, unread)
        warm_sb = consts.tile([128, TJ], BF16, tag="warm", name="warm", bufs=1)
        nc.vector.memset(warm_sb[:], 1.0)
        for wi in range(36):
            wps = psA.tile([128, TJ], F32, tag="mm512", bufs=2, name=f"warm{wi}")
            nc.tensor.matmul(wps[:], warm_sb[:, 0:128], warm_sb[:],
                             start=True, stop=True)

        # inline p0 projections for j=0, rest queued
        push_qk_chain(0, 0)
        require(("xk", 0, 0))
        push_proj(0, first_p_inline=True)

        # ---------------------------------------------------------------
        # Main loop
        # ---------------------------------------------------------------
        for j in range(NJ):
            if j == 0:
                pass  # proj(1) pushed mid-att(0), after its stream DMAs
            elif j + 1 < NJ:
                push_proj(j + 1)

            rcat_cur[0] = [
                work.tile([128, TJ], BF16, tag=f"rcat{t}", bufs=2, name=f"rcat{t}_{j}")
                for t in range(2)
            ]
            if j < 2:
                # unused rows stay 1.0 forever (ln->0, exp->1; selector zeroes
                # them), so only the first two buffer generations need memset
                for t in range(2):
                    nc.vector.memset(rcat_cur[0][t][:], 1.0)
            osb_cur[0] = [
                work.tile([128, TJ], BF16, tag=f"osb{p}", bufs=2, name=f"osb{p}_{j}")
                for p in range(NP)
            ]
            osc_sb = osc_sb_all[j % 2]
            rcat_sb = rcat_cur[0]
            osb_sb = osb_cur[0]

            for p in range(NP):
                if p == 1:
                    if j == 0:
                        # j=1 streams + wp deferred out of the startup window
                        alloc_stream_tiles(1)
                        emit_stream_dmas(1)
                        for pp in range(NP):
                            nc.sync.dma_start(
                                out=wp_sb[pp][:],
                                in_=wpT[pp * 128 : (pp + 1) * 128, :])
                        push_proj(1)
                    release_pending()
                emit_attention(p, j)

            # all proj(j) closures are drained by now (att(j) p=3 required
            # ("xk", 3, j)), so the buffer-recycling DMA is safe to emit
            if j + 2 < NJ:
                alloc_stream_tiles(j + 2)
                emit_stream_dmas(j + 2)

            push_tail(j, rcat_sb, osb_sb, osc_sb)
            push_y(j, osc_sb)

        # drain everything left (tail + y of the last tiles)
        release_pending()
        while fillers:
            _run_one()

    _split_multi_waits(nc)
    return nc


# one-hot row selector: col block pp (=p%2) picks rows 32*(2*pp) and
# 32*(2*pp+1) for output partitions 0-63 / 64-127
_SELEXP = np.zeros((128, 256), NPBF16)
for _pp in range(2):
    _SELEXP[32 * (2 * _pp), _pp * 128 : _pp * 128 + 64] = 1.0
    _SELEXP[32 * (2 * _pp + 1), _pp * 128 + 64 : _pp * 128 + 128] = 1.0

_CACHE = {}


def _get_program(mask):
    key = np.asarray(mask, dtype=bool).tobytes()
    prog = _CACHE.get(key)
    if prog is None:
        _install_patches()
        btab, patterns = _classify_mask(mask)
        nc = _build(btab, len(patterns))
        prog = (nc, patterns)
        _CACHE[key] = prog
    return prog


def _prepare(k, q, v, mask, Wk, Wq, Wv, Wp):
    """Build (cached) the SPMD program and the 8 per-core input maps."""
    k = np.asarray(k, np.float32)
    q = np.asarray(q, np.float32)
    v = np.asarray(v, np.float32)
    Wk = np.asarray(Wk, np.float32)
    Wq = np.asarray(Wq, np.float32)
    Wv = np.asarray(Wv, np.float32)
    Wp = np.asarray(Wp, np.float32)

    nc, patterns = _get_program(mask)
    patflat = np.ascontiguousarray(patterns.reshape(-1, 128))

    def tr_tiled(x, dt):
        # [T, E] f32 -> partition-major [128, NJ*EC*TJ]: partition p holds
        # x^T[e*128+p, j*TJ+c] at col ((j*EC+e)*TJ + c) -> contiguous 1-packet
        # per-partition DMAs
        xt = np.ascontiguousarray(x.astype(dt).T)          # [E, T]
        xt = xt.reshape(EC, 128, NJ, TJ).transpose(1, 2, 0, 3)  # [128, NJ, EC, TJ]
        return np.ascontiguousarray(xt.reshape(128, NJ * EC * TJ))

    def wcat(W, half, dt, scale=1.0):
        # [H, E, D] -> partition-major [128, EC*DC] (e-chunk c at cols c*DC)
        w = (W[half * HPC : (half + 1) * HPC].transpose(1, 0, 2)
             .reshape(E, DC) * scale).astype(dt)
        w = w.reshape(EC, 128, DC).transpose(1, 0, 2)
        return np.ascontiguousarray(w.reshape(128, EC * DC))

    in_maps = []
    for c in range(8):
        b, half = divmod(c, 2)
        off = half * DC
        in_maps.append(
            {
                "qTt": tr_tiled(q[b], NPFP8),
                "kTt": tr_tiled(k[b], NPFP8),
                "vTt": tr_tiled(v[b], NPBF16),
                # wq/wk pre-scaled x16 for fp8 range; exp scale divides it out
                "wq": wcat(Wq, half, NPFP8, 16.0),
                "wk": wcat(Wk, half, NPFP8, 16.0),
                "wv": wcat(Wv, half, NPBF16),
                "wpT": np.ascontiguousarray(Wp[:, off : off + DC].T).astype(NPBF16),
                "pat": patflat,
                "selexp": _SELEXP,
            }
        )
    return nc, in_maps


def kernel(k, q, v, mask, Wk, Wq, Wv, Wp, bp):
    bp = np.asarray(bp, np.float32)
    nc, in_maps = _prepare(k, q, v, mask, Wk, Wq, Wv, Wp)
    res = run_bass_kernel_spmd(nc, in_maps, list(range(8)))
    out = np.empty((B, T, E), np.float32)
    for b in range(B):
        yt = (res.results[2 * b]["yTt"].astype(np.float32)
              + res.results[2 * b + 1]["yTt"].astype(np.float32))
        # [NJ*EC*128, TJ] -> [E, T]
        yt = yt.reshape(NJ, EC, 128, TJ).transpose(1, 2, 0, 3).reshape(E, T)
        out[b] = yt.T + bp[None, :]
    return out
